# revision 26
# baseline (speedup 1.0000x reference)
"""GQA attention block (q 32 heads / kv 8 heads, T=2048, C=4096) on 8 trn2
NeuronCores.

Sharding: tensor-parallel over heads only (TP=8). Core c handles q heads
4c..4c+3 and kv head c for BOTH batches; the host sums the 8 row-parallel
partial outputs.

All matmul operands are bf16 (1 PE cycle/row at any free size); PSUM
accumulates fp32. Per (batch, 512-token block) stage s, the emission order
interleaves attention blocks of stage s with Q-projection chains of stage
s+1 and O-projection chains of stage s-1, so the PE queue never drains
while softmax (ACT exp) or normalization (DVE) latencies resolve.

Device-side layouts keep head dim (hs=128) on SBUF partitions:
  Q^T, K^T: [hs, tok]   (projections emit transposed; RoPE rotate-half is
                         an SBUF partition-swap DMA + sign-folded sin)
  V:        [tok, hs]   (projected as V^T then PE-transposed)
  scores^T: [tk, tq]    (exp batched over 2-chunk PSUM groups; softmax
                         denominators via ones-matmul; 1/D via
                         reciprocal_approx_fast)

Optional fp8 (e4m3, DoubleRow) path for the two big projections (Q and O):
weights/x/y carried as fp8 with a x32 scale folded into the exp scale and
the host-side output scale.
"""

import os
import sys

for _p in ("/root/.axon_site", "/root/.axon_site/_ro/trn_rl_repo",
           "/root/.axon_site/_ro/pypackages", "/opt/trn_rl_repo", "/opt/pypackages"):
    if os.path.isdir(_p) and _p not in sys.path:
        sys.path.append(_p)

import numpy as np

import concourse.bass as bass
import concourse.tile as tile
from concourse import mybir
from concourse.bass_utils import run_bass_kernel_spmd

F32 = mybir.dt.float32
BF16 = mybir.dt.bfloat16
F8 = mybir.dt.float8e4

B, T, C = 2, 2048, 4096
H, KVH, HS = 32, 8, 128
ROPE_BASE = 10000.0

NCORES = 8
HL = H // NCORES      # 4 local q heads
CCH = C // 128        # 32 contraction chunks
JB = 512              # token block (tq block and projection block)
NJ = T // JB          # 4
NSTG = B * NJ         # 8 pipeline stages (b, j)
SCALE = float(1.0 / np.sqrt(HS))
W_UP = 32.0           # fp8 weight upscale (folded back out)


def _split1(nc, max_waits=1):
    """Split instructions with >1 sem wait into preceding NOPs (the cayman
    CTRL codegen only accepts one sync-wait command per instruction)."""
    n = 0
    for f in nc.m.functions:
        for bb in f.blocks:
            out = []
            for inst in bb.instructions:
                si = inst.sync_info
                if si is not None and si.on_wait and len(si.on_wait) > max_waits:
                    w = list(si.on_wait)
                    chunks = [w[i:i + max_waits] for i in range(0, len(w), max_waits)]
                    for j, ch in enumerate(chunks[:-1]):
                        out.append(mybir.InstNoOp(
                            name=f"{inst.name}-wsplit{j}", engine=inst.engine,
                            ins=[], outs=[],
                            sync_info=mybir.SyncInfo(on_wait=ch, on_update=[])))
                        n += 1
                    inst.sync_info = mybir.SyncInfo(
                        on_wait=chunks[-1], on_update=list(si.on_update))
                out.append(inst)
            bb.instructions[:] = out
    return n


def build_nc(with_bias=False, fp8=False, split=True):
    nc = bass.Bass("TRN2")
    P = 128
    QDT = F8 if fp8 else BF16     # xq / wq operand dtype
    YDT = F8 if fp8 else BF16     # yT / wo operand dtype

    # --- DRAM parameters (per-core shards, host-pre-tiled layouts) ---
    dp = nc.declare_dram_parameter
    xq_d = dp("xq", [P, B, NJ, CCH, JB], QDT, isOutput=False)   # x^T tiles
    xkv_d = dp("xkv", [P, B, NJ, CCH, JB], BF16, isOutput=False)
    wq_d = dp("wq", [P, CCH, HL * HS], QDT, isOutput=False)     # [ki, ko, n]
    wkv_d = dp("wkv", [P, CCH, 2 * HS], BF16, isOutput=False)   # [K | V]
    wo_d = dp("wo", [P, HL, C], YDT, isOutput=False)            # [ki, chl, n]
    cosT_d = dp("cosT", [HS, T], BF16, isOutput=False)
    sinT_d = dp("sinT", [HS, T], BF16, isOutput=False)          # sign-folded
    ident_d = dp("ident", [P, P], BF16, isOutput=False)
    ones_d = dp("ones", [P, P], BF16, isOutput=False)           # 1/W_UP if fp8
    mask_d = dp("mask", [P, P + JB], BF16, isOutput=False)      # mult. causal
    if with_bias:
        bq_d = dp("bq", [HL * HS], F32, isOutput=False)
        bkv_d = dp("bkv", [2 * HS], F32, isOutput=False)
    out_d = dp("out", [B, T, C], BF16, isOutput=True)

    with tile.TileContext(nc) as tc:
        with (
            tc.tile_pool(name="consts", bufs=1) as consts,
            tc.tile_pool(name="kvres", bufs=1) as kvres,
            tc.tile_pool(name="wper", bufs=1) as wper,
            tc.tile_pool(name="ropep", bufs=2) as ropep,
            tc.tile_pool(name="xqp", bufs=3) as xqp,
            tc.tile_pool(name="pp_proj", bufs=2, space="PSUM") as pp_proj,
        ):
            ident_sb = consts.tile([P, P], BF16)
            ones_sb = consts.tile([P, P], BF16)
            mask_sb = consts.tile([P, P + JB], BF16)
            cos_sb = consts.tile([HS, T], BF16)
            sin_sb = consts.tile([HS, T], BF16)
            nc.sync.dma_start(out=ident_sb, in_=ident_d[:])
            nc.sync.dma_start(out=ones_sb, in_=ones_d[:])
            nc.sync.dma_start(out=mask_sb, in_=mask_d[:])
            nc.sync.dma_start(out=cos_sb, in_=cosT_d[:])
            nc.sync.dma_start(out=sin_sb, in_=sinT_d[:])
            if with_bias:
                bq_sb = consts.tile([P, HL], F32)
                bkv_sb = consts.tile([P, 2], F32)
                nc.sync.dma_start(out=bq_sb, in_=bq_d[:].rearrange("(h p) -> p h", p=P))
                nc.sync.dma_start(out=bkv_sb, in_=bkv_d[:].rearrange("(h p) -> p h", p=P))

            kT_sb = kvres.tile([HS, B, T], BF16)            # [hs, b, tok]
            v_sb = kvres.tile([P, B * NJ * 4, HS], BF16)    # [tok, chunk, hs]
            wq_sb = wper.tile([P, CCH, HL * HS], QDT)

            def rope(dst, ps, ts, bias_col=None):
                """dst[hs, JB] = rope(ps + bias); ts = token slice."""
                src = ropep.tile([P, JB], BF16, tag="src")
                if bias_col is not None:
                    nc.vector.tensor_scalar(
                        out=src, in0=ps, scalar1=bias_col, scalar2=None,
                        op0=mybir.AluOpType.add)
                else:
                    nc.scalar.copy(out=src, in_=ps)
                rot = ropep.tile([P, JB], BF16, tag="rot")
                nc.sync.dma_start(out=rot[0:64, :], in_=src[64:128, :])
                nc.sync.dma_start(out=rot[64:128, :], in_=src[0:64, :])
                tmp = ropep.tile([P, JB], BF16, tag="tmp")
                nc.vector.tensor_mul(tmp, rot, sin_sb[:, ts])
                nc.vector.tensor_mul(dst, src, cos_sb[:, ts])
                nc.vector.tensor_add(dst, dst, tmp)

            def load_x_halves(xp, x_d, b, j, tag):
                xa = xp.tile([P, CCH // 2, JB], x_d.dtype, tag=tag)
                xb = xp.tile([P, CCH // 2, JB], x_d.dtype, tag=tag)
                nc.sync.dma_start(out=xa, in_=x_d[:, b, j, :CCH // 2])
                nc.sync.dma_start(out=xb, in_=x_d[:, b, j, CCH // 2:])
                return xa, xb

            xq_t = {}

            # ---------------- P1: K^T / V^T projections ----------------
            QCH = CCH // 4            # 8 ko chunks per xkv quarter-tile
            with (
                tc.tile_pool(name="wkvp", bufs=1) as wkvp,
                tc.tile_pool(name="vstg", bufs=2) as vstg,
                tc.tile_pool(name="xkvp", bufs=8) as xkvp,
                tc.tile_pool(name="pp_t", bufs=2, space="PSUM") as pp_t,
            ):
                wkv_sb = wkvp.tile([P, CCH, 2 * HS], BF16)
                blocks = [(b, j) for b in range(B) for j in range(NJ)]
                xkv_t = {}

                def load_xkv_q(i, q):
                    b, j = blocks[i]
                    xt = xkvp.tile([P, QCH, JB], BF16, tag="xkv",
                                   name=f"xkv{i}q{q}")
                    nc.sync.dma_start(
                        out=xt, in_=xkv_d[:, b, j, q * QCH:(q + 1) * QCH])
                    xkv_t.setdefault(i, []).append(xt)

                # prologue: x quarters on the sync DGE ring; weight loads on
                # the scalar DGE ring so the two streams don't serialize
                load_xkv_q(0, 0)
                nc.scalar.dma_start(out=wkv_sb[:, :CCH // 2], in_=wkv_d[:, :CCH // 2])
                load_xkv_q(0, 1)
                nc.scalar.dma_start(out=wkv_sb[:, CCH // 2:], in_=wkv_d[:, CCH // 2:])
                load_xkv_q(0, 2)
                load_xkv_q(0, 3)
                for q in range(4):
                    load_xkv_q(1, q)

                # staggered big loads, one slot consumed per KV block
                def wq_quarter(q):
                    return lambda: nc.scalar.dma_start(
                        out=wq_sb[:, q * QCH:(q + 1) * QCH],
                        in_=wq_d[:, q * QCH:(q + 1) * QCH])

                def xq0_half(k):
                    def go():
                        if 0 not in xq_t:
                            xq_t[0] = []
                        xa = xqp.tile([P, CCH // 2, JB], xq_d.dtype, tag="xq",
                                      name=f"xq0h{k}")
                        nc.sync.dma_start(
                            out=xa,
                            in_=xq_d[:, 0, 0, k * (CCH // 2):(k + 1) * (CCH // 2)])
                        xq_t[0].append(xa)
                    return go

                staggered = [wq_quarter(0), wq_quarter(1), wq_quarter(2),
                             wq_quarter(3), xq0_half(0), xq0_half(1)]

                for i, (b, j) in enumerate(blocks):
                    if i + 2 < len(blocks):
                        for q in range(4):
                            load_xkv_q(i + 2, q)
                    if staggered:
                        staggered.pop(0)()
                    xqs = xkv_t[i]
                    ts = slice(j * JB, (j + 1) * JB)
                    # K chain
                    ps = pp_proj.tile([P, JB], F32, tag="proj")
                    for c in range(CCH):
                        nc.tensor.matmul(
                            ps, lhsT=wkv_sb[:, c, 0:P],
                            rhs=xqs[c // QCH][:, c % QCH, :],
                            start=(c == 0), stop=(c == CCH - 1))
                    rope(kT_sb[:, b, ts], ps, ts,
                         bkv_sb[:, 0:1] if with_bias else None)
                    # V chain (V^T then PE-transpose to [tok, hs])
                    ps2 = pp_proj.tile([P, JB], F32, tag="proj")
                    for c in range(CCH):
                        nc.tensor.matmul(
                            ps2, lhsT=wkv_sb[:, c, P:2 * P],
                            rhs=xqs[c // QCH][:, c % QCH, :],
                            start=(c == 0), stop=(c == CCH - 1))
                    vt = vstg.tile([P, JB], BF16, tag="vt")
                    if with_bias:
                        nc.vector.tensor_scalar(
                            out=vt, in0=ps2, scalar1=bkv_sb[:, 1:2],
                            scalar2=None, op0=mybir.AluOpType.add)
                    else:
                        nc.scalar.copy(out=vt, in_=ps2)
                    for sub in range(JB // P):
                        pt = pp_t.tile([P, P], BF16, tag="tp")
                        nc.tensor.transpose(
                            pt, vt[:, sub * P:(sub + 1) * P], ident_sb)
                        nc.vector.tensor_copy(
                            out=v_sb[:, b * 16 + j * 4 + sub, :], in_=pt)
                    del xkv_t[i]

            # -------- P2/P3/P4: Q proj + attention + O proj pipeline --------
            with (
                tc.tile_pool(name="pbuf", bufs=4) as pbuf,
                tc.tile_pool(name="rbuf", bufs=2) as rbuf,
                tc.tile_pool(name="ostg", bufs=3) as ostg,
                tc.tile_pool(name="padp", bufs=3) as padp,
                tc.tile_pool(name="wop", bufs=1) as wop,
                tc.tile_pool(name="qtp", bufs=2) as qtp,
                tc.tile_pool(name="ytp", bufs=2) as ytp,
                tc.tile_pool(name="pp_s", bufs=2, space="PSUM") as pp_s,
                tc.tile_pool(name="pp_acc", bufs=2, space="PSUM") as pp_acc,
            ):
                escale = SCALE / W_UP if fp8 else SCALE
                wo_sb = wop.tile([P, HL, C], YDT)

                qT_t = [qtp.tile([HS, HL, JB], BF16, tag="qT", name=f"qT{i}")
                        for i in range(2)]
                yT_t = [ytp.tile([HS, HL, JB], YDT, tag="yT", name=f"yT{i}")
                        for i in range(2)]

                def qproj_chain(s, h):
                    b, j = divmod(s, NJ)
                    if h == 0 and s not in xq_t:
                        xq_t[s] = load_x_halves(xqp, xq_d, b, j, tag="xq")
                    xa, xb = xq_t[s]
                    ps = pp_proj.tile([P, JB], F32, tag="proj")
                    if fp8:
                        for c2 in range(CCH // 2):
                            xt = xa if c2 < CCH // 4 else xb
                            c2l = c2 % (CCH // 4)
                            nc.tensor.matmul(
                                ps, lhsT=wq_sb[:, 2 * c2:2 * c2 + 2,
                                               h * HS:(h + 1) * HS],
                                rhs=xt[:, 2 * c2l:2 * c2l + 2, :],
                                start=(c2 == 0), stop=(c2 == CCH // 2 - 1),
                                perf_mode=mybir.MatmulPerfMode.DoubleRow)
                    else:
                        for c in range(CCH):
                            xt = xa if c < CCH // 2 else xb
                            nc.tensor.matmul(
                                ps, lhsT=wq_sb[:, c, h * HS:(h + 1) * HS],
                                rhs=xt[:, c % (CCH // 2), :],
                                start=(c == 0), stop=(c == CCH - 1))
                    ts = slice((s % NJ) * JB, (s % NJ + 1) * JB)
                    rope(qT_t[s % 2][:, h, :], ps, ts,
                         bq_sb[:, h:h + 1] if with_bias else None)

                def attn_block(s, h):
                    b, j = divmod(s, NJ)
                    qb = qT_t[s % 2][:, h, :]
                    ps_y = pp_acc.tile([P, JB], F32, tag="acc")
                    ps_sum = pp_acc.tile([P, JB], F32, tag="acc")
                    ng = 2 * (j + 1)

                    def group_desc(g):
                        # two tk chunks (a, a+1) packed contiguously: chunk k
                        # at psum/pT column col_k, covering tq window [off_k:]
                        descs, col = [], 0
                        for a in (2 * g, 2 * g + 1):
                            m = a - 4 * j
                            off = P * m if m > 0 else 0
                            w = JB - off
                            descs.append((a, col, off, w))
                            col += w
                        return descs

                    def flush(g, pT, padd, last):
                        for a, col, off, w in group_desc(g):
                            lastc = last and (a == 2 * g + 1)
                            nc.tensor.matmul(
                                ps_y[:, off:], lhsT=v_sb[:, b * 16 + a, :],
                                rhs=pT[:, col:col + w],
                                start=(a == 0), stop=lastc)
                            if padd is None:
                                nc.tensor.matmul(
                                    ps_sum[:, off:], lhsT=ones_sb,
                                    rhs=pT[:, col:col + w],
                                    start=(a == 0), stop=lastc)
                        if padd is not None:
                            # non-diag pair pre-added on DVE: one ones-matmul
                            nc.tensor.matmul(
                                ps_sum, lhsT=ones_sb, rhs=padd,
                                start=(g == 0), stop=last)

                    pend = []
                    for g in range(ng):
                        ps_s = pp_s.tile([P, 2 * JB], F32, tag="s")
                        descs = group_desc(g)
                        for a, col, off, w in descs:
                            nc.tensor.matmul(
                                ps_s[:, col:col + w],
                                lhsT=kT_sb[:, b, a * P:(a + 1) * P],
                                rhs=qb[:, off:], start=True, stop=True)
                        span = descs[-1][1] + descs[-1][3]
                        if len(pend) == 2:
                            flush(*pend.pop(0), last=False)
                        pT = pbuf.tile([P, 2 * JB], BF16, tag="pT")
                        nc.scalar.activation(
                            out=pT[:, :span], in_=ps_s[:, :span],
                            func=mybir.ActivationFunctionType.Exp, scale=escale)
                        padd = None
                        if 2 * g + 1 < 4 * j:   # fully non-diagonal group
                            padd = padp.tile([P, JB], BF16, tag="padd")
                            nc.vector.tensor_add(padd, pT[:, :JB], pT[:, JB:])
                        else:
                            for a, col, off, w in descs:
                                if a - 4 * j >= 0:   # diagonal: triangle mask
                                    nc.vector.tensor_mul(
                                        pT[:, col:col + w],
                                        pT[:, col:col + w],
                                        mask_sb[:, P:P + w])
                        pend.append((g, pT, padd))
                    while pend:
                        flush(*pend.pop(0), last=(len(pend) == 0))
                    lnd = rbuf.tile([P, JB], F32, tag="lnd")
                    nc.scalar.activation(
                        out=lnd, in_=ps_sum,
                        func=mybir.ActivationFunctionType.Ln)
                    rec = rbuf.tile([P, JB], F32, tag="rec")
                    nc.scalar.activation(
                        out=rec, in_=lnd,
                        func=mybir.ActivationFunctionType.Exp, scale=-1.0)
                    nc.vector.tensor_mul(yT_t[s % 2][:, h, :], ps_y, rec)

                def oproj_chains(s, h_slot):
                    b, j = divmod(s, NJ)
                    yT = yT_t[s % 2]
                    for o in (2 * h_slot, 2 * h_slot + 1):
                        for i in range(JB // P):
                            ps = pp_proj.tile([P, JB], F32, tag="proj")
                            if fp8:
                                for p2 in range(HL // 2):
                                    nc.tensor.matmul(
                                        ps,
                                        lhsT=yT[:, 2 * p2:2 * p2 + 2,
                                                i * P:(i + 1) * P],
                                        rhs=wo_sb[:, 2 * p2:2 * p2 + 2,
                                                  o * JB:(o + 1) * JB],
                                        start=(p2 == 0),
                                        stop=(p2 == HL // 2 - 1),
                                        perf_mode=mybir.MatmulPerfMode.DoubleRow)
                            else:
                                for chl in range(HL):
                                    nc.tensor.matmul(
                                        ps, lhsT=yT[:, chl, i * P:(i + 1) * P],
                                        rhs=wo_sb[:, chl, o * JB:(o + 1) * JB],
                                        start=(chl == 0), stop=(chl == HL - 1))
                            ot = ostg.tile([P, JB], BF16, tag="ot")
                            if (o + i) % 2 == 0:
                                nc.scalar.copy(out=ot, in_=ps)
                            else:
                                nc.vector.tensor_copy(out=ot, in_=ps)
                            r0 = j * JB + i * P
                            dma_eng = nc.sync if (o + i) % 2 == 0 else nc.scalar
                            dma_eng.dma_start(
                                out=out_d[b, r0:r0 + P, o * JB:(o + 1) * JB],
                                in_=ot)

                for h in range(HL):
                    qproj_chain(0, h)
                for s in range(NSTG):
                    for h in range(HL):
                        attn_block(s, h)
                        if s + 1 < NSTG:
                            qproj_chain(s + 1, h)
                        if s == 0:
                            # stagger wo quarters into the DMA queue; quarter
                            # h is exactly what oproj slot h consumes
                            nc.scalar.dma_start(
                                out=wo_sb[:, :, h * C // 4:(h + 1) * C // 4],
                                in_=wo_d[:, :, h * C // 4:(h + 1) * C // 4])
                        if s >= 1:
                            oproj_chains(s - 1, h)
                for h in range(HL):
                    oproj_chains(NSTG - 1, h)

    if split:
        _split1(nc)
    return nc


def _rope_tables():
    inv_freq = (1.0 / (np.float32(ROPE_BASE) **
                       (np.arange(0, HS, 2, dtype=np.float32) / np.float32(HS))))
    pos = np.arange(T, dtype=np.float32)
    ang = pos[:, None] * inv_freq[None, :]
    ang = np.concatenate([ang, ang], axis=-1).astype(np.float32)  # [T, HS]
    return np.cos(ang).astype(np.float32), np.sin(ang).astype(np.float32)


def _tile_x(x, dt):
    # [B, T, C] -> [ki=128, b, j, ko, tok]  (x^T tiled, contraction-major)
    return np.ascontiguousarray(
        x.reshape(B, NJ, JB, CCH, 128).transpose(4, 0, 1, 3, 2)).astype(dt)


def _tile_w(w, dt):
    # [C, N] -> [ki=128, ko, N]
    n = w.shape[1]
    return np.ascontiguousarray(
        w.reshape(CCH, 128, n).transpose(1, 0, 2)).astype(dt)


def _consts():
    ident = np.eye(128, dtype=np.float32)
    ones = np.ones((128, 128), np.float32)
    u = np.arange(128 + JB)[None, :]
    i = np.arange(128)[:, None]
    mask = (u >= i + 128).astype(np.float32)
    return ident, ones, mask


_NC_CACHE = {}


def make_in_maps(inp, with_bias, fp8=False):
    q_x, kv_x = inp["q_x"], inp["kv_x"]
    Wq, Wk, Wv, Wo = inp["Wq"], inp["Wk"], inp["Wv"], inp["Wo"]
    cos, sin = _rope_tables()
    bf16 = mybir.dt.np(BF16)
    f8 = mybir.dt.np(F8)
    qdt = f8 if fp8 else bf16
    cosT = np.ascontiguousarray(cos.T).astype(bf16)   # [HS, T]
    sinT = np.ascontiguousarray(sin.T).copy()
    sinT[:64, :] *= -1.0                              # sign of rotate-half
    sinT = sinT.astype(bf16)
    ident, ones, mask = _consts()
    if fp8:
        ones = ones / W_UP
    xq_all = _tile_x(q_x, qdt)
    xkv_all = _tile_x(kv_x, bf16)
    in_maps = []
    for core in range(NCORES):
        g = core
        wq = Wq[:, g * HL * HS:(g + 1) * HL * HS]
        wo = Wo[g * HL * HS:(g + 1) * HL * HS, :]
        if fp8:
            wq = wq * W_UP
            wo = wo * W_UP
        m = {
            "xq": xq_all,
            "xkv": xkv_all,
            "wq": _tile_w(wq, qdt),
            "wkv": _tile_w(np.concatenate(
                [Wk[:, g * HS:(g + 1) * HS],
                 Wv[:, g * HS:(g + 1) * HS]], axis=1), bf16),
            "wo": np.ascontiguousarray(
                wo.reshape(HL, 128, C).transpose(1, 0, 2)).astype(
                    f8 if fp8 else bf16),
            "cosT": cosT, "sinT": sinT,
            "ident": ident.astype(bf16), "ones": ones.astype(bf16),
            "mask": mask.astype(bf16),
        }
        if with_bias:
            m["bq"] = np.ascontiguousarray(inp["bq"][g * HL * HS:(g + 1) * HL * HS])
            m["bkv"] = np.ascontiguousarray(
                np.stack([inp["bk"][g * HS:(g + 1) * HS],
                          inp["bv"][g * HS:(g + 1) * HS]]).reshape(-1))
        in_maps.append(m)
    return in_maps


FP8 = False


def kernel(**inputs):
    inp = {k: np.asarray(v, dtype=np.float32) for k, v in inputs.items()}
    with_bias = bool(np.any(inp["bq"]) or np.any(inp["bk"]) or np.any(inp["bv"]))
    key = ("nc", with_bias, FP8)
    if key not in _NC_CACHE:
        _NC_CACHE[key] = build_nc(with_bias=with_bias, fp8=FP8)
    nc = _NC_CACHE[key]

    in_maps = make_in_maps(inp, with_bias, fp8=FP8)
    res = run_bass_kernel_spmd(nc, in_maps, list(range(NCORES)))
    out = np.zeros((B, T, C), np.float32)
    for core in range(NCORES):
        out += res.results[core]["out"].astype(np.float32)
    if FP8:
        out /= (W_UP * W_UP)
    out += inp["bo"]
    return out
